# revision 13
# baseline (speedup 1.0000x reference)
"""Attention-pooling kernel for TRN2 (8 NeuronCores, batch-parallel).

Computes, for x:[32,2048,1024], W:[1024,1024], b:[1024], ctx:[1024]:
    h = tanh(x @ W + b); scores = h . ctx
    weights = softmax(scores, axis=seq)
    out = sum_s weights[s] * x[s]          -> [32, 1024]

Sharding: data-parallel over batch, 4 batches per core.

x and W are cast to fp16 on the host (10-bit mantissa keeps the score
error at the f32r-baseline level) so the kernel can use the xbar DMA
transpose: each 512-row seq sub-tile of x is transposed HBM -> SBUF by
the DMA engines directly into a [128, 8, 512] tile with
xT[p, k, s] = x[s, 128k + p], i.e. each embed-block k is a contiguous
moving operand for the PE. No PE transposes, no PSUM-evacuation
copies, and x is read from HBM exactly once (pass 2 pools from the
same transposed tiles on the Vector engine).

PE stream: per 512-seq sub-tile, 64 fp16 matmuls (full rate) + the 8
ctx-dot matmuls of the PREVIOUS sub-tile as one contiguous batch
(mid-stream stationary/bank switches cost ~2x95ns on HW, so they are
not interleaved). Pass 2 runs on otherwise-idle engines: softmax exp
on ScalarE (row layout, accum gives Z), weight broadcast via tiny K=1
matmuls, pooling as fused multiply-reduce (tensor_tensor_reduce) on
DVE against the transposed tiles, staged across the next batch's
sub-tiles so the PE never waits on the softmax chain.
"""

import numpy as np
from contextlib import ExitStack

import concourse.bacc as bacc
import concourse.mybir as mybir
import concourse.tile as tile
from concourse import masks
from concourse.bass_utils import run_bass_kernel_spmd

B, S, E, A = 32, 2048, 1024, 1024
NCORES = 8
BL = B // NCORES          # batches per core
ST = 512                  # seq sub-tile
NSUB = S // ST            # sub-tiles per batch
KE = E // 128             # contraction chunks over embed dim
KA = A // 128             # chunks over attention dim

F32 = mybir.dt.float32
F16 = mybir.dt.float16
AX = mybir.AxisListType.X
AF = mybir.ActivationFunctionType
ALU = mybir.AluOpType


def _build(reps=1):
    nc = bacc.Bacc("TRN2", target_bir_lowering=False, debug=False,
                   num_devices=NCORES)
    x_d = nc.declare_dram_parameter("x", [BL * S, E], F16, isOutput=False)
    W_d = nc.declare_dram_parameter("W", [E, A], F16, isOutput=False)
    b_d = nc.declare_dram_parameter("b", [A], F32, isOutput=False)
    c_d = nc.declare_dram_parameter("ctx", [A], F16, isOutput=False)
    o_d = nc.declare_dram_parameter("out", [BL, E], F32, isOutput=True)

    with ExitStack() as ctx:
        tc = ctx.enter_context(tile.TileContext(nc))

        const_pool = ctx.enter_context(tc.tile_pool(name="const", bufs=1))
        xt_pool = ctx.enter_context(tc.tile_pool(name="xT", bufs=9))
        h_pool = ctx.enter_context(tc.tile_pool(name="h", bufs=12))
        sc_pool = ctx.enter_context(tc.tile_pool(name="scores", bufs=2))
        sm_pool = ctx.enter_context(tc.tile_pool(name="softmax", bufs=2))
        tt_pool = ctx.enter_context(tc.tile_pool(name="ttr", bufs=2))
        out_pool = ctx.enter_context(tc.tile_pool(name="outs", bufs=1))

        ps_h = ctx.enter_context(tc.tile_pool(name="ps_h", bufs=2, space="PSUM"))
        ps_s = ctx.enter_context(tc.tile_pool(name="ps_s", bufs=2, space="PSUM"))
        ps_t = ctx.enter_context(tc.tile_pool(name="ps_t", bufs=2, space="PSUM"))
        ps_w = ctx.enter_context(tc.tile_pool(name="ps_w", bufs=2, space="PSUM"))

        # ---- constants ----
        ident = const_pool.tile([128, 128], F32)
        masks.make_identity(nc, ident[:])
        ones128 = const_pool.tile([1, 128], F32)
        nc.gpsimd.memset(ones128[:], 1.0)
        ones16 = const_pool.tile([1, 128], F16)
        nc.gpsimd.memset(ones16[:], 1.0)

        W_sb = const_pool.tile([128, KE * A], F16)
        b_sb = const_pool.tile([128, KA], F32)
        ctx_sb = const_pool.tile([128, KA], F16)

        tiles = [(rep, bi, t)
                 for rep in range(reps) for bi in range(BL) for t in range(NSUB)]

        def dma_xt(bi, t):
            # xbar DMA transpose: x rows [512, 1024] -> [128, 8, 512] with
            # xT[p, k, s] = x[r0 + s, 128k + p] (the k dim is "logically part
            # of the partition dim": out logical column e = 128k + p)
            r0 = bi * S + t * ST
            xT = xt_pool.tile([128, KE, ST], F16, tag="xT")
            nc.sync.dma_start_transpose(xT[:], x_d[r0:r0 + ST, :])
            return xT

        # ---- pass 2, staged across the next batch's sub-tiles ----

        def flush_a(st):
            # softmax chain part 1: ScalarE/DVE only, no PE involvement
            scores_sb = st["scores"]
            m_sb = sm_pool.tile([1, 1], F32, tag="m")
            nc.vector.reduce_max(m_sb[:], st["pmax"][:], axis=AX)
            negm = sm_pool.tile([1, 1], F32, tag="negm")
            nc.vector.tensor_scalar_mul(negm[:], m_sb[:], -1.0)
            w16row = sm_pool.tile([1, S], F16, tag="w16row")
            zparts = sm_pool.tile([1, NSUB], F32, tag="zparts")
            for t in range(NSUB):
                nc.scalar.activation(w16row[0:1, t * ST:(t + 1) * ST],
                                     scores_sb[0:1, t * ST:(t + 1) * ST],
                                     AF.Exp, bias=negm[0:1, 0:1],
                                     accum_out=zparts[0:1, t:t + 1])
            z_sb = sm_pool.tile([1, 1], F32, tag="z")
            nc.vector.reduce_sum(z_sb[:], zparts[:], axis=AX)
            st["w16row"], st["z"] = w16row, z_sb

        def flush_b(st):
            # 1/Z for the final 8 output partitions (tiny K=1 matmul), then
            # broadcast the exp'd weights across partitions and pool on DVE
            # with fused multiply-reduce against the transposed x tiles
            zb_ps = ps_t.tile([128, 1], F32, tag="tps")
            nc.tensor.matmul(zb_ps[:], ones128[:], st["z"][:],
                             start=True, stop=True)
            z128 = sm_pool.tile([128, 1], F32, tag="z128")
            nc.scalar.activation(z128[:], zb_ps[:], AF.Copy)
            rz = sm_pool.tile([128, 1], F32, tag="rz")
            nc.vector.reciprocal(rz[:], z128[:])
            st["rz"] = rz

            w16row = st["w16row"]
            wB = sm_pool.tile([128, S], F16, tag="wB")
            for t in range(NSUB):
                wb_ps = ps_w.tile([128, ST], F32, tag="wbps")
                nc.tensor.matmul(wb_ps[:], ones16[:],
                                 w16row[0:1, t * ST:(t + 1) * ST],
                                 start=True, stop=True)
                nc.scalar.activation(wB[:, t * ST:(t + 1) * ST], wb_ps[:],
                                     AF.Copy)

            part = sm_pool.tile([128, KE * NSUB], F32, tag="part")
            for k in range(KE):
                for t in range(NSUB):
                    scratch = tt_pool.tile([128, ST], F16, tag="ttr")
                    nc.vector.tensor_tensor(
                        scratch[:], st["xts"][t][:, k, :],
                        wB[:, t * ST:(t + 1) * ST], ALU.mult)
                    nc.vector.reduce_sum(
                        part[:, k * NSUB + t:k * NSUB + t + 1], scratch[:],
                        axis=AX)
            pooled = sm_pool.tile([128, KE], F32, tag="pooled")
            for k in range(KE):
                nc.vector.reduce_sum(pooled[:, k:k + 1],
                                     part[:, k * NSUB:(k + 1) * NSUB], axis=AX)
            # normalize by 1/Z with a per-partition scalar (HW-proven path)
            pooledn = sm_pool.tile([128, KE], F32, tag="pooledn")
            nc.vector.tensor_scalar_mul(pooledn[:], pooled[:],
                                        st["rz"][:, 0:1])
            st["pooled"] = pooledn

        def flush_c(st):
            # transpose [128 e-low, 8 k] -> [8 k, 128 e-low], scale by 1/Z
            # on the way out, and store the row
            ot_ps = ps_t.tile([KE, 128], F32, tag="tps")
            nc.tensor.transpose(ot_ps[:], st["pooled"][:], ident[:])
            ot_sb = out_pool.tile([KE, 128], F32, tag="ot")
            nc.scalar.activation(ot_sb[:], ot_ps[:], AF.Copy)
            nc.sync.dma_start(
                o_d[st["row"]:st["row"] + 1, :].rearrange(
                    "r (k e) -> (r k) e", e=128),
                ot_sb[:])

        # prologue: first xT transposes go out before the W load so the
        # first matmul group can start ASAP
        xT_cur = dma_xt(tiles[0][1], tiles[0][2])
        for k in range(KE):
            nc.sync.dma_start(W_sb[:, k * A:(k + 1) * A],
                              W_d[k * 128:(k + 1) * 128, :])
        nc.sync.dma_start(b_sb[:], b_d.rearrange("(j p) -> p j", p=128))
        nc.sync.dma_start(ctx_sb[:], c_d.rearrange("(j p) -> p j", p=128))
        xT_next = dma_xt(tiles[1][1], tiles[1][2])

        # warm the PE HAM clock-gate with throwaway matmuls while the first
        # DMAs land (the PE would otherwise idle cold and re-throttle)
        warm_scratch = out_pool.tile([128, 512], F32, tag="warm")
        for w in range(28):
            wp = ps_h.tile([128, 128], F32, tag="hps", name=f"warm{w}")
            nc.tensor.matmul(wp[:], ident[:], ident[:], start=True, stop=True)
            if w % 14 == 13:
                nc.scalar.activation(warm_scratch[:, 0:128], wp[:], AF.Copy)

        pending = None
        scores_sb = None
        batch_xts = []
        ctx_q = []          # deferred ctx-dot matmuls (one sub-tile behind)

        for i, (rep, bi, t) in enumerate(tiles):
            if t == 0:
                scores_sb = sc_pool.tile([1, S], F32, tag="scores")
                pmax_sb = sc_pool.tile([1, NSUB], F32, tag="pmax")
                batch_xts = []
            batch_xts.append(xT_cur)

            if pending is not None:
                if t == 1:
                    flush_a(pending)
                elif t == 2:
                    flush_b(pending)
                elif t == 3:
                    flush_c(pending)
                    pending = None

            # prefetch the transposed tile two sub-tiles ahead
            if i + 2 < len(tiles):
                xT_pre = dma_xt(tiles[i + 2][1], tiles[i + 2][2])
            else:
                xT_pre = None

            sc_ps = ps_s.tile([1, ST], F32, tag="scps")
            for j in range(KA):
                hp = ps_h.tile([128, ST], F32, tag="hps")
                for k in range(KE):
                    nc.tensor.matmul(
                        hp[:],
                        W_sb[:, k * A + j * 128: k * A + (j + 1) * 128],
                        xT_cur[:, k, :],
                        start=(k == 0), stop=(k == KE - 1))
                # drain the whole previous sub-tile's ctx-dot as ONE
                # contiguous group (all its tanhs are long done)
                if j == 1:
                    while ctx_q:
                        ctx_q.pop(0)()
                h_sb = h_pool.tile([128, ST], F16, tag="h")
                nc.scalar.activation(h_sb[:], hp[:], AF.Tanh,
                                     bias=b_sb[:, j:j + 1])
                ctx_q.append(
                    lambda j=j, h_sb=h_sb, sc_ps=sc_ps: nc.tensor.matmul(
                        sc_ps[:], ctx_sb[:, j:j + 1], h_sb[:],
                        start=(j == 0), stop=(j == KA - 1)))
            # the ctx-dot group drains during the next sub-tile; the score
            # copy/max ride behind it in the deferred queue so they are
            # emitted only after the accumulation group is closed
            def copy_scores(scores_sb=scores_sb, pmax_sb=pmax_sb,
                            sc_ps=sc_ps, t=t):
                nc.vector.tensor_copy(
                    scores_sb[:, t * ST:(t + 1) * ST], sc_ps[:])
                nc.vector.reduce_max(pmax_sb[:, t:t + 1], sc_ps[:], axis=AX)
            ctx_q.append(copy_scores)
            if i == len(tiles) - 1:
                while ctx_q:
                    ctx_q.pop(0)()

            if t == NSUB - 1:
                pending = {"scores": scores_sb, "pmax": pmax_sb,
                           "xts": list(batch_xts), "row": bi}

            xT_cur, xT_next = xT_next, xT_pre

        if pending is not None:
            flush_a(pending)
            flush_b(pending)
            flush_c(pending)

    nc.compile()
    return nc


_NC_CACHE = None


def kernel(x, W, b, ctx):
    global _NC_CACHE
    if _NC_CACHE is None:
        _NC_CACHE = _build()
    nc = _NC_CACHE

    x16 = np.ascontiguousarray(np.asarray(x).astype(np.float16))
    W16 = np.ascontiguousarray(np.asarray(W).astype(np.float16))
    b = np.ascontiguousarray(np.asarray(b, dtype=np.float32))
    c16 = np.ascontiguousarray(np.asarray(ctx).astype(np.float16))

    in_maps = [
        {"x": x16[i * BL:(i + 1) * BL].reshape(BL * S, E), "W": W16, "b": b,
         "ctx": c16}
        for i in range(NCORES)
    ]
    res = run_bass_kernel_spmd(nc, in_maps, core_ids=list(range(NCORES)))
    return np.concatenate([res.results[i]["out"] for i in range(NCORES)],
                          axis=0)


if __name__ == "__main__":
    rng = np.random.default_rng(0)
    x = rng.standard_normal((B, S, E), dtype=np.float32)
    W = rng.standard_normal((E, A), dtype=np.float32) / np.sqrt(E)
    b = rng.standard_normal((A,), dtype=np.float32) * 0.01
    c = rng.standard_normal((A,), dtype=np.float32)
    out = kernel(x=x, W=W, b=b, ctx=c)
    print(out.shape, out.dtype)


# revision 17
# speedup vs baseline: 1.0837x; 1.0837x over previous
"""Attention-pooling kernel for TRN2 (8 NeuronCores, batch-parallel).

Computes, for x:[32,2048,1024], W:[1024,1024], b:[1024], ctx:[1024]:
    h = tanh(x @ W + b); scores = h . ctx
    weights = softmax(scores, axis=seq)
    out = sum_s weights[s] * x[s]          -> [32, 1024]

Sharding: data-parallel over batch, 4 batches per core.

x and W are cast to fp16 on the host (10-bit mantissa keeps the score
error at the f32r-baseline level) so the kernel can use the xbar DMA
transpose: each 512-row seq sub-tile of x is transposed HBM -> SBUF by
the DMA engines directly into a [128, 8, 512] tile with
xT[p, k, s] = x[s, 128k + p], i.e. each embed-block k is a contiguous
moving operand for the PE. No PE transposes, no PSUM-evacuation
copies, and x is read from HBM exactly once (pooling reads the same
transposed tiles on the Vector engine).

PE stream: per 512-seq sub-tile, 64 fp16 matmuls (full rate), then the
previous sub-tile's 8 ctx-dot matmuls as one contiguous group
(mid-stream stationary/bank switches cost ~2x95ns on HW) plus one K=1
broadcast matmul. The softmax+pooling is ONLINE (flash-attention
style): each sub-tile is pooled immediately with provisional weights
exp(s - m_t) (fused multiply-reduce on the otherwise-idle DVE, against
the transposed tiles), and a per-batch scalar fixup r_t = exp(m_t - m)
/ Z rescales the four partial pools at the end — so nothing big ever
serializes behind the last scores.
"""

import numpy as np
from contextlib import ExitStack

import concourse.bacc as bacc
import concourse.mybir as mybir
import concourse.tile as tile
from concourse import masks
from concourse.bass_utils import run_bass_kernel_spmd

B, S, E, A = 32, 2048, 1024, 1024
NCORES = 8
BL = B // NCORES          # batches per core
ST = 512                  # seq sub-tile
NSUB = S // ST            # sub-tiles per batch
KE = E // 128             # contraction chunks over embed dim
KA = A // 128             # chunks over attention dim

F32 = mybir.dt.float32
F16 = mybir.dt.float16
AX = mybir.AxisListType.X
AF = mybir.ActivationFunctionType
ALU = mybir.AluOpType


def _build(reps=1):
    nc = bacc.Bacc("TRN2", target_bir_lowering=False, debug=False,
                   num_devices=NCORES)
    x_d = nc.declare_dram_parameter("x", [BL * S, E], F16, isOutput=False)
    W_d = nc.declare_dram_parameter("W", [E, A], F16, isOutput=False)
    b_d = nc.declare_dram_parameter("b", [A], F32, isOutput=False)
    c_d = nc.declare_dram_parameter("ctx", [A], F16, isOutput=False)
    o_d = nc.declare_dram_parameter("out", [BL, E], F32, isOutput=True)

    with ExitStack() as ctx:
        tc = ctx.enter_context(tile.TileContext(nc))

        const_pool = ctx.enter_context(tc.tile_pool(name="const", bufs=1))
        xt_pool = ctx.enter_context(tc.tile_pool(name="xT", bufs=6))
        h_pool = ctx.enter_context(tc.tile_pool(name="h", bufs=12))
        sc_pool = ctx.enter_context(tc.tile_pool(name="scores", bufs=2))
        sm_pool = ctx.enter_context(tc.tile_pool(name="softmax", bufs=2))
        pp_pool = ctx.enter_context(tc.tile_pool(name="parts", bufs=5))
        tt_pool = ctx.enter_context(tc.tile_pool(name="ttr", bufs=2))
        out_pool = ctx.enter_context(tc.tile_pool(name="outs", bufs=2))

        ps_h = ctx.enter_context(tc.tile_pool(name="ps_h", bufs=2, space="PSUM"))
        ps_s = ctx.enter_context(tc.tile_pool(name="ps_s", bufs=2, space="PSUM"))
        ps_t = ctx.enter_context(tc.tile_pool(name="ps_t", bufs=2, space="PSUM"))
        ps_w = ctx.enter_context(tc.tile_pool(name="ps_w", bufs=2, space="PSUM"))

        # ---- constants ----
        ident = const_pool.tile([128, 128], F32)
        masks.make_identity(nc, ident[:])
        ones128 = const_pool.tile([1, 128], F32)
        nc.gpsimd.memset(ones128[:], 1.0)
        ones16 = const_pool.tile([1, 128], F16)
        nc.gpsimd.memset(ones16[:], 1.0)

        W_sb = const_pool.tile([128, KE * A], F16)
        b_sb = const_pool.tile([128, KA], F32)
        ctx_sb = const_pool.tile([128, KA], F16)

        tiles = [(rep, bi, t)
                 for rep in range(reps) for bi in range(BL) for t in range(NSUB)]

        def dma_xt(bi, t):
            # xbar DMA transpose: x rows [512, 1024] -> [128, 8, 512] with
            # xT[p, k, s] = x[r0 + s, 128k + p] (the k dim is "logically part
            # of the partition dim": out logical column e = 128k + p)
            r0 = bi * S + t * ST
            xT = xt_pool.tile([128, KE, ST], F16, tag="xT")
            nc.sync.dma_start_transpose(xT[:], x_d[r0:r0 + ST, :])
            return xT

        # ---- online-softmax partial pooling, one sub-tile behind pass 1 ----

        def softmax_partial(st):
            # provisional weights for this sub-tile: exp(s - m_t); DVE/ACT
            # only, no PE involvement
            t, sc_ps = st["t"], st["sc_ps"]
            mcat, zcat = st["mcat"], st["zcat"]
            nc.vector.reduce_max(mcat[0:1, t:t + 1], sc_ps[:], axis=AX)
            negm = sm_pool.tile([1, 1], F32, tag="negm")
            nc.vector.tensor_scalar_mul(negm[:], mcat[0:1, t:t + 1], -1.0)
            w16 = sm_pool.tile([1, ST], F16, tag="w16")
            nc.scalar.activation(w16[:], sc_ps[:], AF.Exp, bias=negm[0:1, 0:1],
                                 accum_out=zcat[0:1, t:t + 1])
            st["w16"] = w16

        def pool_partial(st):
            # broadcast the weights across partitions (K=1 matmul) and pool
            # this sub-tile on DVE with fused multiply-reduce against the
            # transposed x tile
            wb_ps = ps_w.tile([128, ST], F32, tag="wbps")
            nc.tensor.matmul(wb_ps[:], ones16[:], st["w16"][:],
                             start=True, stop=True)
            wB = sm_pool.tile([128, ST], F16, tag="wB")
            nc.scalar.activation(wB[:], wb_ps[:], AF.Copy)
            part = pp_pool.tile([128, KE], F32, tag="part")
            for k in range(KE):
                scratch = tt_pool.tile([128, ST], F16, tag="ttr")
                nc.vector.tensor_tensor(
                    scratch[:], st["xT"][:, k, :], wB[:], ALU.mult)
                nc.vector.reduce_sum(part[:, k:k + 1], scratch[:], axis=AX)
            st["parts"].append(part)

        def flush_dve(st):
            # combine the 4 partial pools: r_t = exp(m_t - m) / Z
            mcat, zcat = st["mcat"], st["zcat"]
            m_sb = sm_pool.tile([1, 1], F32, tag="m")
            nc.vector.reduce_max(m_sb[:], mcat[:], axis=AX)
            negm = sm_pool.tile([1, 1], F32, tag="negmb")
            nc.vector.tensor_scalar_mul(negm[:], m_sb[:], -1.0)
            r4 = sm_pool.tile([1, NSUB], F32, tag="r4")
            nc.scalar.activation(r4[:], mcat[:], AF.Exp, bias=negm[0:1, 0:1])
            rz4 = sm_pool.tile([1, NSUB], F32, tag="rz4")
            nc.vector.tensor_tensor(rz4[:], r4[:], zcat[:], ALU.mult)
            z_sb = sm_pool.tile([1, 1], F32, tag="z")
            nc.vector.reduce_sum(z_sb[:], rz4[:], axis=AX)
            iz = sm_pool.tile([1, 1], F32, tag="iz")
            nc.vector.reciprocal(iz[:], z_sb[:])
            rn4 = sm_pool.tile([1, NSUB], F32, tag="rn4")
            nc.vector.tensor_scalar_mul(rn4[:], r4[:], iz[0:1, 0:1])
            st["rn4"] = rn4

        def flush_pe(st):
            # broadcast r_t/Z to all partitions, rescale+sum the partials,
            # transpose [128 e-low, 8 k] -> [8 k, 128 e-low], store the row
            rb_ps = ps_t.tile([128, NSUB], F32, tag="tps")
            nc.tensor.matmul(rb_ps[:], ones128[:], st["rn4"][:],
                             start=True, stop=True)
            rB = sm_pool.tile([128, NSUB], F32, tag="rB")
            nc.scalar.activation(rB[:], rb_ps[:], AF.Copy)
            acc = []
            for t in range(NSUB):
                sc = sm_pool.tile([128, KE], F32, tag=f"psc{t}")
                nc.vector.tensor_scalar_mul(sc[:], st["parts"][t][:],
                                            rB[:, t:t + 1])
                acc.append(sc)
            a01 = sm_pool.tile([128, KE], F32, tag="a01")
            nc.vector.tensor_tensor(a01[:], acc[0][:], acc[1][:], ALU.add)
            a23 = sm_pool.tile([128, KE], F32, tag="a23")
            nc.vector.tensor_tensor(a23[:], acc[2][:], acc[3][:], ALU.add)
            pooled = sm_pool.tile([128, KE], F32, tag="pooled")
            nc.vector.tensor_tensor(pooled[:], a01[:], a23[:], ALU.add)
            ot_ps = ps_t.tile([KE, 128], F32, tag="tps")
            nc.tensor.transpose(ot_ps[:], pooled[:], ident[:])
            ot_sb = out_pool.tile([KE, 128], F32, tag="ot")
            nc.scalar.activation(ot_sb[:], ot_ps[:], AF.Copy)
            nc.sync.dma_start(
                o_d[st["row"]:st["row"] + 1, :].rearrange(
                    "r (k e) -> (r k) e", e=128),
                ot_sb[:])

        # prologue: first xT transposes go out before the W load so the
        # first matmul group can start ASAP
        xT_cur = dma_xt(tiles[0][1], tiles[0][2])
        for k in range(KE):
            nc.sync.dma_start(W_sb[:, k * A:(k + 1) * A],
                              W_d[k * 128:(k + 1) * 128, :])
        nc.sync.dma_start(b_sb[:], b_d.rearrange("(j p) -> p j", p=128))
        nc.sync.dma_start(ctx_sb[:], c_d.rearrange("(j p) -> p j", p=128))
        xT_next = dma_xt(tiles[1][1], tiles[1][2])

        # warm the PE HAM clock-gate with throwaway matmuls while the first
        # DMAs land (the PE would otherwise idle cold and re-throttle)
        warm_scratch = out_pool.tile([128, 512], F32, tag="warm")
        for w in range(28):
            wp = ps_h.tile([128, 128], F32, tag="hps", name=f"warm{w}")
            nc.tensor.matmul(wp[:], ident[:], ident[:], start=True, stop=True)
            if w % 14 == 13:
                nc.scalar.activation(warm_scratch[:, 0:128], wp[:], AF.Copy)

        # deferred-work queues, drained between the NEXT sub-tile's matmul
        # groups so the PE never waits on the ScalarE/DVE chains
        q1, q3, q5 = [], [], []
        batch = None

        def drain(q):
            while q:
                q.pop(0)()

        for i, (rep, bi, t) in enumerate(tiles):
            if t == 0:
                batch = {"row": bi,
                         "mcat": sc_pool.tile([1, NSUB], F32, tag="mcat",
                                              name=f"mcat{i}"),
                         "zcat": sc_pool.tile([1, NSUB], F32, tag="zcat",
                                              name=f"zcat{i}"),
                         "parts": []}

            # prefetch the transposed tile two sub-tiles ahead
            if i + 2 < len(tiles):
                xT_pre = dma_xt(tiles[i + 2][1], tiles[i + 2][2])
            else:
                xT_pre = None

            st = dict(batch, t=t, xT=xT_cur, parts=batch["parts"])
            sc_ps = ps_s.tile([1, ST], F32, tag="scps")
            st["sc_ps"] = sc_ps

            for j in range(KA):
                hp = ps_h.tile([128, ST], F32, tag="hps")
                for k in range(KE):
                    nc.tensor.matmul(
                        hp[:],
                        W_sb[:, k * A + j * 128: k * A + (j + 1) * 128],
                        xT_cur[:, k, :],
                        start=(k == 0), stop=(k == KE - 1))
                if j == 1:
                    drain(q1)
                elif j == 3:
                    drain(q3)
                elif j == 5:
                    drain(q5)
                h_sb = h_pool.tile([128, ST], F16, tag="h")
                nc.scalar.activation(h_sb[:], hp[:], AF.Tanh,
                                     bias=b_sb[:, j:j + 1])
                q1.append(
                    lambda j=j, h_sb=h_sb, sc_ps=sc_ps: nc.tensor.matmul(
                        sc_ps[:], ctx_sb[:, j:j + 1], h_sb[:],
                        start=(j == 0), stop=(j == KA - 1)))
            # previous-sub-tile work rides the deferred queues: the ctx-dot
            # group closes at the next sub-tile's j==1, softmax right after,
            # the broadcast+pooling at j==3, the batch fixup at j==3/j==5
            q1.append(lambda st=st: softmax_partial(st))
            q3.append(lambda st=st: pool_partial(st))
            if t == NSUB - 1:
                q3.append(lambda st=st: flush_dve(st))
                q5.append(lambda st=st: flush_pe(st))

            xT_cur, xT_next = xT_next, xT_pre

        drain(q1)
        drain(q3)
        drain(q5)

    nc.compile()
    return nc


_NC_CACHE = None


def kernel(x, W, b, ctx):
    global _NC_CACHE
    if _NC_CACHE is None:
        _NC_CACHE = _build()
    nc = _NC_CACHE

    x16 = np.ascontiguousarray(np.asarray(x).astype(np.float16))
    W16 = np.ascontiguousarray(np.asarray(W).astype(np.float16))
    b = np.ascontiguousarray(np.asarray(b, dtype=np.float32))
    c16 = np.ascontiguousarray(np.asarray(ctx).astype(np.float16))

    in_maps = [
        {"x": x16[i * BL:(i + 1) * BL].reshape(BL * S, E), "W": W16, "b": b,
         "ctx": c16}
        for i in range(NCORES)
    ]
    res = run_bass_kernel_spmd(nc, in_maps, core_ids=list(range(NCORES)))
    return np.concatenate([res.results[i]["out"] for i in range(NCORES)],
                          axis=0)


if __name__ == "__main__":
    rng = np.random.default_rng(0)
    x = rng.standard_normal((B, S, E), dtype=np.float32)
    W = rng.standard_normal((E, A), dtype=np.float32) / np.sqrt(E)
    b = rng.standard_normal((A,), dtype=np.float32) * 0.01
    c = rng.standard_normal((A,), dtype=np.float32)
    out = kernel(x=x, W=W, b=b, ctx=c)
    print(out.shape, out.dtype)


# revision 19
# speedup vs baseline: 1.1746x; 1.0839x over previous
"""Attention-pooling kernel for TRN2 (8 NeuronCores, batch-parallel).

Computes, for x:[32,2048,1024], W:[1024,1024], b:[1024], ctx:[1024]:
    h = tanh(x @ W + b); scores = h . ctx
    weights = softmax(scores, axis=seq)
    out = sum_s weights[s] * x[s]          -> [32, 1024]

Sharding: data-parallel over batch, 4 batches per core.

x and W are cast to fp16 on the host (10-bit mantissa keeps the score
error at the f32r-baseline level) so the kernel can use the xbar DMA
transpose: each 512-row seq sub-tile of x is transposed HBM -> SBUF by
the DMA engines directly into a [128, 8, 512] tile with
xT[p, k, s] = x[s, 128k + p], i.e. each embed-block k is a contiguous
moving operand for the PE — no PE transposes and no PSUM-evacuation
copies for pass 1.

All heavy math stays on the PE: offloading the pooling to DVE was
tried and REGRESSED (~20% slower matmuls): big [128,512] DVE reads
contend with the PE's moving-operand SBUF fetches, so idle engines are
not free. Instead the softmax+pooling is ONLINE (flash-attention
style): each sub-tile's scores are transposed (4 tiny PE matmuls),
exponentiated against the sub-tile max (GPSIMD cross-partition max,
128-wide ACT exp), and pooled immediately (8 PE matmuls against a
straight fp16 copy of x), one sub-tile behind pass 1. A per-batch
scalar fixup r_t = exp(m_t - m)/Z rescales the four partial pools, so
nothing big serializes behind the last scores. Mid-stream PE
stationary/bank switches cost ~2x95ns on HW, so deferred work drains
as contiguous same-shape groups between pass-1 matmul groups.
"""

import numpy as np
from contextlib import ExitStack

import concourse.bacc as bacc
import concourse.mybir as mybir
import concourse.tile as tile
from concourse import masks
from concourse.bass_isa import ReduceOp
from concourse.bass_utils import run_bass_kernel_spmd

B, S, E, A = 32, 2048, 1024, 1024
NCORES = 8
BL = B // NCORES          # batches per core
ST = 512                  # seq sub-tile
NSUB = S // ST            # sub-tiles per batch
NCH = ST // 128           # 128-row s-chunks per sub-tile
KE = E // 128             # contraction chunks over embed dim
KA = A // 128             # chunks over attention dim

F32 = mybir.dt.float32
F16 = mybir.dt.float16
AX = mybir.AxisListType.X
AF = mybir.ActivationFunctionType
ALU = mybir.AluOpType


def _build(reps=1):
    nc = bacc.Bacc("TRN2", target_bir_lowering=False, debug=False,
                   num_devices=NCORES)
    x_d = nc.declare_dram_parameter("x", [BL * S, E], F16, isOutput=False)
    W_d = nc.declare_dram_parameter("W", [E, A], F16, isOutput=False)
    b_d = nc.declare_dram_parameter("b", [A], F32, isOutput=False)
    c_d = nc.declare_dram_parameter("ctx", [A], F16, isOutput=False)
    o_d = nc.declare_dram_parameter("out", [BL, E], F32, isOutput=True)

    with ExitStack() as ctx:
        tc = ctx.enter_context(tile.TileContext(nc))

        const_pool = ctx.enter_context(tc.tile_pool(name="const", bufs=1))
        xt_pool = ctx.enter_context(tc.tile_pool(name="xT", bufs=3))
        xb_pool = ctx.enter_context(tc.tile_pool(name="xb", bufs=3))
        h_pool = ctx.enter_context(tc.tile_pool(name="h", bufs=12))
        sc_pool = ctx.enter_context(tc.tile_pool(name="scores", bufs=2))
        sm_pool = ctx.enter_context(tc.tile_pool(name="softmax", bufs=2))
        pp_pool = ctx.enter_context(tc.tile_pool(name="parts", bufs=5))
        out_pool = ctx.enter_context(tc.tile_pool(name="outs", bufs=2))

        ps_h = ctx.enter_context(tc.tile_pool(name="ps_h", bufs=2, space="PSUM"))
        ps_s = ctx.enter_context(tc.tile_pool(name="ps_s", bufs=2, space="PSUM"))
        ps_t = ctx.enter_context(tc.tile_pool(name="ps_t", bufs=2, space="PSUM"))
        ps_o = ctx.enter_context(tc.tile_pool(name="ps_o", bufs=1, space="PSUM"))

        # ---- constants ----
        ident = const_pool.tile([128, 128], F32)
        masks.make_identity(nc, ident[:])

        W_sb = const_pool.tile([128, KE * A], F16)
        b_sb = const_pool.tile([128, KA], F32)
        ctx_sb = const_pool.tile([128, KA], F16)

        tiles = [(rep, bi, t)
                 for rep in range(reps) for bi in range(BL) for t in range(NSUB)]

        def dma_xt(bi, t):
            # xbar DMA transpose: x rows [512, 1024] -> [128, 8, 512] with
            # xT[p, k, s] = x[r0 + s, 128k + p]
            r0 = bi * S + t * ST
            xT = xt_pool.tile([128, KE, ST], F16, tag="xT")
            nc.sync.dma_start_transpose(xT[:], x_d[r0:r0 + ST, :])
            return xT

        def dma_xb(bi, t):
            # straight fp16 copy for the pooling moving operand:
            # xb[p, c, e] = x[r0 + 128c + p, e]
            r0 = bi * S + t * ST
            xb = xb_pool.tile([128, NCH, E], F16, tag="xb")
            nc.sync.dma_start(
                xb[:], x_d[r0:r0 + ST, :].rearrange("(c p) e -> p c e", p=128))
            return xb

        # ---- online softmax + pooling, one sub-tile behind pass 1 ----

        def scores_T(st):
            # PE: raw scores row [1,512] -> [128, 4] via 4 tiny transposes
            sc_row = sm_pool.tile([1, ST], F32, tag="srow")
            nc.vector.tensor_copy(sc_row[:], st["sc_ps"][:])
            tp = ps_t.tile([128, NCH], F32, tag="tps")
            for u in range(NCH):
                nc.tensor.matmul(
                    tp[:, u:u + 1], sc_row[0:1, u * 128:(u + 1) * 128],
                    ident[0:1, 0:1], is_transpose=True,
                    start=(u == 0), stop=(u == NCH - 1),
                    skip_group_check=True)
            st["tp"] = tp

        def softmax_partial(st):
            # GPSIMD cross-partition max + 128-wide exp; zcat partials stay
            # per-partition until the batch fixup
            t, tp = st["t"], st["tp"]
            mc = sm_pool.tile([128, 1], F32, tag="mc")
            nc.vector.reduce_max(mc[:], tp[:], axis=AX)
            nc.gpsimd.partition_all_reduce(st["mcat"][:, t:t + 1], mc[:], 128,
                                           ReduceOp.max)
            negm = sm_pool.tile([128, 1], F32, tag="negm")
            nc.vector.tensor_scalar_mul(negm[:], st["mcat"][:, t:t + 1], -1.0)
            pT = sm_pool.tile([128, NCH], F16, tag="pT")
            nc.scalar.activation(pT[:], tp[:], AF.Exp, bias=negm[:, 0:1],
                                 accum_out=st["zcat"][:, t:t + 1])
            st["pT"] = pT

        def pool_partial(st):
            # PE pooling of this sub-tile with its provisional weights
            pT, xb = st["pT"], st["xb"]
            op0 = ps_o.tile([1, 512], F32, tag="op0")
            op1 = ps_o.tile([1, 512], F32, tag="op1")
            for c in range(NCH):
                nc.tensor.matmul(op0[:], pT[:, c:c + 1], xb[:, c, 0:512],
                                 start=(c == 0), stop=(c == NCH - 1))
            for c in range(NCH):
                nc.tensor.matmul(op1[:], pT[:, c:c + 1], xb[:, c, 512:1024],
                                 start=(c == 0), stop=(c == NCH - 1))
            part = pp_pool.tile([1, E], F32, tag="part",
                                name=f"part{st['i']}")
            nc.scalar.activation(part[0:1, 0:512], op0[:], AF.Copy)
            nc.scalar.activation(part[0:1, 512:1024], op1[:], AF.Copy)
            st["parts"].append(part)

        def flush(st):
            # per-batch fixup: r_t = exp(m_t - m) / Z, all [128,*]-wide and
            # tiny; then rescale+sum the four [1, E] partial pools
            mcat, zcat = st["mcat"], st["zcat"]
            mG = sm_pool.tile([128, 1], F32, tag="mG")
            nc.vector.reduce_max(mG[:], mcat[:], axis=AX)
            negG = sm_pool.tile([128, 1], F32, tag="negG")
            nc.vector.tensor_scalar_mul(negG[:], mG[:], -1.0)
            r4 = sm_pool.tile([128, NSUB], F32, tag="r4")
            nc.scalar.activation(r4[:], mcat[:], AF.Exp, bias=negG[:, 0:1])
            rz4 = sm_pool.tile([128, NSUB], F32, tag="rz4")
            nc.vector.tensor_tensor(rz4[:], r4[:], zcat[:], ALU.mult)
            zp = sm_pool.tile([128, 1], F32, tag="zp")
            nc.vector.reduce_sum(zp[:], rz4[:], axis=AX)
            z_all = sm_pool.tile([128, 1], F32, tag="zall")
            nc.gpsimd.partition_all_reduce(z_all[:], zp[:], 128, ReduceOp.add)
            iz = sm_pool.tile([128, 1], F32, tag="iz")
            nc.vector.reciprocal(iz[:], z_all[:])
            rn4 = sm_pool.tile([128, NSUB], F32, tag="rn4")
            nc.vector.tensor_scalar_mul(rn4[:], r4[:], iz[:, 0:1])

            parts = st["parts"]
            sc = []
            for t in range(NSUB):
                s_t = sm_pool.tile([1, E], F32, tag=f"psc{t}")
                nc.vector.tensor_scalar_mul(s_t[:], parts[t][:],
                                            rn4[0:1, t:t + 1])
                sc.append(s_t)
            a01 = sm_pool.tile([1, E], F32, tag="a01")
            nc.vector.tensor_tensor(a01[:], sc[0][:], sc[1][:], ALU.add)
            a23 = sm_pool.tile([1, E], F32, tag="a23")
            nc.vector.tensor_tensor(a23[:], sc[2][:], sc[3][:], ALU.add)
            orow = out_pool.tile([1, E], F32, tag="orow")
            nc.vector.tensor_tensor(orow[:], a01[:], a23[:], ALU.add)
            nc.sync.dma_start(o_d[st["row"]:st["row"] + 1, :], orow[:])

        # prologue: first xT transposes go out before the W load so the
        # first matmul group can start ASAP
        xT_cur = dma_xt(tiles[0][1], tiles[0][2])
        for k in range(KE):
            nc.sync.dma_start(W_sb[:, k * A:(k + 1) * A],
                              W_d[k * 128:(k + 1) * 128, :])
        nc.sync.dma_start(b_sb[:], b_d.rearrange("(j p) -> p j", p=128))
        nc.sync.dma_start(ctx_sb[:], c_d.rearrange("(j p) -> p j", p=128))
        xT_next = dma_xt(tiles[1][1], tiles[1][2])
        xb_cur = dma_xb(tiles[0][1], tiles[0][2])

        # warm the PE HAM clock-gate with throwaway matmuls while the first
        # DMAs land (the PE would otherwise idle cold and re-throttle)
        warm_scratch = out_pool.tile([128, 512], F32, tag="warm")
        for w in range(28):
            wp = ps_h.tile([128, 128], F32, tag="hps", name=f"warm{w}")
            nc.tensor.matmul(wp[:], ident[:], ident[:], start=True, stop=True)
            if w % 14 == 13:
                nc.scalar.activation(warm_scratch[:, 0:128], wp[:], AF.Copy)

        # deferred-work queues, drained between the NEXT sub-tile's matmul
        # groups so the PE never waits on the ScalarE/DVE chains
        q1, q3, q5 = [], [], []
        batch = None

        def drain(q):
            while q:
                q.pop(0)()

        for i, (rep, bi, t) in enumerate(tiles):
            if t == 0:
                batch = {"row": bi,
                         "mcat": sc_pool.tile([128, NSUB], F32, tag="mcat",
                                              name=f"mcat{i}"),
                         "zcat": sc_pool.tile([128, NSUB], F32, tag="zcat",
                                              name=f"zcat{i}"),
                         "parts": []}

            # prefetch: transposed tile two sub-tiles ahead, straight tile
            # one ahead
            if i + 2 < len(tiles):
                xT_pre = dma_xt(tiles[i + 2][1], tiles[i + 2][2])
            else:
                xT_pre = None
            if i + 1 < len(tiles):
                xb_next = dma_xb(tiles[i + 1][1], tiles[i + 1][2])
            else:
                xb_next = None

            st = dict(batch, i=i, t=t, xb=xb_cur)
            sc_ps = ps_s.tile([1, ST], F32, tag="scps")
            st["sc_ps"] = sc_ps

            for j in range(KA):
                hp = ps_h.tile([128, ST], F32, tag="hps")
                for k in range(KE):
                    nc.tensor.matmul(
                        hp[:],
                        W_sb[:, k * A + j * 128: k * A + (j + 1) * 128],
                        xT_cur[:, k, :],
                        start=(k == 0), stop=(k == KE - 1))
                if j == 1:
                    drain(q1)
                elif j == 3:
                    drain(q3)
                elif j == 5:
                    drain(q5)
                h_sb = h_pool.tile([128, ST], F16, tag="h")
                nc.scalar.activation(h_sb[:], hp[:], AF.Tanh,
                                     bias=b_sb[:, j:j + 1])
                q1.append(
                    lambda j=j, h_sb=h_sb, sc_ps=sc_ps: nc.tensor.matmul(
                        sc_ps[:], ctx_sb[:, j:j + 1], h_sb[:],
                        start=(j == 0), stop=(j == KA - 1)))
            # previous-sub-tile work rides the deferred queues: ctx-dot
            # group and score transpose at the next sub-tile's j==1, exp at
            # j==3, pooling at j==5, the batch fixup one sub-tile later
            q1.append(lambda st=st: scores_T(st))
            q3.append(lambda st=st: softmax_partial(st))
            q5.append(lambda st=st: pool_partial(st))
            if t == NSUB - 1:
                q5.append(lambda st=st: flush(st))

            xT_cur, xT_next = xT_next, xT_pre
            xb_cur = xb_next

        drain(q1)
        drain(q3)
        drain(q5)

    nc.compile()
    return nc


_NC_CACHE = None


def kernel(x, W, b, ctx):
    global _NC_CACHE
    if _NC_CACHE is None:
        _NC_CACHE = _build()
    nc = _NC_CACHE

    x16 = np.ascontiguousarray(np.asarray(x).astype(np.float16))
    W16 = np.ascontiguousarray(np.asarray(W).astype(np.float16))
    b = np.ascontiguousarray(np.asarray(b, dtype=np.float32))
    c16 = np.ascontiguousarray(np.asarray(ctx).astype(np.float16))

    in_maps = [
        {"x": x16[i * BL:(i + 1) * BL].reshape(BL * S, E), "W": W16, "b": b,
         "ctx": c16}
        for i in range(NCORES)
    ]
    res = run_bass_kernel_spmd(nc, in_maps, core_ids=list(range(NCORES)))
    return np.concatenate([res.results[i]["out"] for i in range(NCORES)],
                          axis=0)


if __name__ == "__main__":
    rng = np.random.default_rng(0)
    x = rng.standard_normal((B, S, E), dtype=np.float32)
    W = rng.standard_normal((E, A), dtype=np.float32) / np.sqrt(E)
    b = rng.standard_normal((A,), dtype=np.float32) * 0.01
    c = rng.standard_normal((A,), dtype=np.float32)
    out = kernel(x=x, W=W, b=b, ctx=c)
    print(out.shape, out.dtype)
